# revision 14
# baseline (speedup 1.0000x reference)
"""DoRA linear layer on 8 TRN2 NeuronCores.

out = (magnitude / ||W + s*B@A||_row) * (x @ (W + s*B@A)^T),  s = alpha/rank = 2.

Identity used: the reference's
    dora_out + base_out = mag_norm_scale * (base_out + s * lora_out)
                        = scale_o * (x @ W_adapted^T)

Sharding: TENSOR-PARALLEL on out_dim: core k owns output columns
[512k, 512(k+1)), x replicated (streamed), W/lora_b/magnitude column-sharded.
Norm/scale computation is fully LOCAL to each core.

Schedule notes (the PE executes its queue in FIFO order, so emission order IS
the schedule; measured constants from the NTFF profile):
  * ~8.7us fixed DMA dead time at kernel start, then ~420GB/s aggregate.
  * W^T ships partition-major (8KiB DMA lines, 4 chunks of 8 blocks) instead
    of 32 x [128, OC] blocks with 1KiB lines -- the baseline's W DMA trickled
    until t=31.7us and starved the aux phase.
  * The W_ad adds (PSUM fp32 + W fp16 -> fp16) are DVE-bound at ~677ns per
    [128,512] block (PSUM reads run 1x mode).  They are batched 2 blocks per
    op over two-bank lora PSUM tiles (~1.16us/pair) and form the aux
    critical chain; everything else pipelines around them:
      - rank-16 LoRA matmuls 4-way ROW-packed (tile_position=(32j,0), K=16),
      - nsq ones-matmuls 4-way COLUMN-packed (tile_position=(0,32j), M=1),
      - W_ad squares batched on the SCALAR engine (activation Square),
      - per aux round r the PE block [lora(r+1) | main(r) | nsq(r)] is
        emitted after the adds of round r, so adds for round r+1 always run
        one round ahead of the PE and the PE never head-of-line blocks.
  * Token tiles t0-t2 join the aux rounds adaptively (t0 at r0, t1 at r2,
    t2 at r4, with catch-up bursts) so the PE stays fed while xb tiles
    arrive; the sync-queue DMA order (aux, wc0, xb0, wc1, xb1, wc2, xb2,
    wc3, xb3..) matches that schedule.
  * scale = mag/sqrt(nsq) broadcasts to [128, OC] via a K=1 ones matmul (no
    DRAM round trip); the 4 nsq strips are reduced with partition-aligned
    PSUM->SBUF copies + one ones-matmul (DVE lanes cannot cross partitions).
  * PSUM budget 8 banks: 2x2 lora ring + 3 main ring + 1 nsq/scale; after
    the aux phase tiles t3-t6 run as half-chains in the freed lora tiles,
    giving ~27us of drain slack so the scale path never stalls the PE.
  * Input DMAs issue on the sync queue, output stores on the scalar queue;
    the last tile's drain is split in halves to shorten the tail.

Host side only reshapes/transposes (layout prep), casts fp32 -> fp16
(accuracy budget is rel_err < 2e-2; fp16 gives ~4e-4), and concatenates
the per-core output column blocks.
"""

import sys

sys.path.insert(0, "/opt/trn_rl_repo")

import numpy as np

import concourse.bass as bass  # noqa: F401  (import keeps bass registered)
from concourse import bacc
import concourse.mybir as mybir
from concourse.tile import TileContext
from concourse.bass_utils import run_bass_kernel_spmd

FP32 = mybir.dt.float32
FP16 = mybir.dt.float16

NCORES = 8
TOK = 8192          # 4 * 2048 tokens total, all processed by every core
DIN = 4096
DOUT = 4096
RANK = 16
SCALING = 32.0 / 16

NI = DIN // 128      # 32 contraction blocks
OC = DOUT // NCORES  # 512 output columns per core
NB = TOK // 128      # 64 token tiles per core
CHUNK_PAIRS = [1, 1, 2, 4, 4, 4]   # W DMA chunk sizes in block-pairs
NR = NI // 4         # 8 aux rounds of 4 blocks
ATW = NR * 128       # atr4 width in aux const
JOIN = {0: 0, 1: 2, 2: 4}   # aux round at which early tile t joins


def _build_program():
    nc = bacc.Bacc("TRN2", target_bir_lowering=False, debug=False,
                   num_devices=NCORES)

    # x in token-block-major layout: block t -> [128 part, NI*128] contiguous
    xb_d = nc.dram_tensor("xb", [NB, 128, NI * 128], FP16,
                          kind="ExternalInput")
    # W^T partition-major: wc[p, ib*OC + o] = W^T[ib*128 + p, o]
    wc_d = nc.dram_tensor("wc", [128, NI * OC], FP16, kind="ExternalInput")
    # aux = atr4 (row-packed A^T) ++ b2n4 (row-packed s*B^T)
    aux_d = nc.dram_tensor("aux", [128, ATW + OC], FP16, kind="ExternalInput")
    magn_d = nc.dram_tensor("magn", [1, OC], FP32, kind="ExternalInput")
    out_d = nc.dram_tensor("out", [TOK, OC], FP32, kind="ExternalOutput")

    with TileContext(nc) as tc:
        with (
            tc.tile_pool(name="const", bufs=1) as const,
            tc.tile_pool(name="xbp", bufs=8) as xbp,
            tc.tile_pool(name="wadp", bufs=NI // 2) as wadp,
            tc.tile_pool(name="wsqp", bufs=2) as wsqp,
            tc.tile_pool(name="outp", bufs=10) as outp,
            tc.tile_pool(name="lorap", bufs=2, space="PSUM") as lorap,
            tc.tile_pool(name="mp", bufs=3, space="PSUM") as mp,
            tc.tile_pool(name="sp", bufs=1, space="PSUM") as sp,
        ):
            # ---- constants ------------------------------------------------
            aux = const.tile([128, ATW + OC], FP16)
            nc.sync.dma_start(aux[:], aux_d[:])
            ones128 = const.tile([128, 1], FP16)
            nc.vector.memset(ones128[:], 1.0)
            onesrow = const.tile([1, 128], FP16)
            nc.vector.memset(onesrow[:], 1.0)
            strip4 = const.tile([128, OC], FP16)
            nc.vector.memset(strip4[:], 0.0)

            # ---- input DMAs (sync queue order == emission order) ----------
            # W chunks lead with small slices so the wad add chain starts as
            # early as possible; xb tiles interleave to match tile joins.
            pair_chunk = []              # pair k -> (chunk tile, local off)
            xb_tiles = {}
            xb_after = {1: 0, 3: 1, 5: 2}   # chunk idx -> xb tile to fetch
            off = 0
            for c, szp in enumerate(CHUNK_PAIRS):
                w_c = const.tile([128, szp * 2 * OC], FP16)
                nc.sync.dma_start(
                    w_c[:], wc_d[:, off * 2 * OC:(off + szp) * 2 * OC])
                for l in range(szp):
                    pair_chunk.append((w_c, l))
                off += szp
                if c in xb_after:
                    t = xb_after[c]
                    xb = xbp.tile([128, NI * 128], FP16, tag="xb",
                                  name=f"xb{t}")
                    nc.sync.dma_start(xb[:], xb_d[t])
                    xb_tiles[t] = xb
            magn_sb = const.tile([1, OC], FP32)
            nc.sync.dma_start(magn_sb[:], magn_d[:])

            # ---- aux rounds interleaved with early-main ------------------
            ps_nsq = sp.tile([128, OC], FP32, tag="sp", name="psnsq")
            # warmup matmuls (gated only on the memsets) keep the PE busy
            # through the aux-DMA window so HAM unthrottles the PE clock
            # before the first real matmul; the real nsq chain's start=True
            # clears their output.
            for i in range(8):
                nc.tensor.matmul(ps_nsq[0:1, :], ones128[:], strip4[:],
                                 start=True, stop=True)
            mains = [mp.tile([128, OC], FP32, tag="mp", name=f"pm{t}")
                     for t in range(3)]
            wad2 = [None] * (NI // 2)    # pair k covers blocks (2k, 2k+1)
            wsq2 = [None] * (NI // 2)
            lora_ps = {}                 # round r -> (tileA, tileB)

            def wad_ap(ib):
                return wad2[ib // 2][:, (ib % 2) * OC:(ib % 2 + 1) * OC]

            def emit_lora_group(r):
                plA = lorap.tile([128, 2 * OC], FP32, tag="pl",
                                 name=f"plA{r}")
                plB = lorap.tile([128, 2 * OC], FP32, tag="pl",
                                 name=f"plB{r}")
                for j in range(4):
                    dst = (plA if j < 2 else plB)
                    nc.tensor.matmul(
                        dst[:, (j % 2) * OC:(j % 2 + 1) * OC],
                        aux[32 * j:32 * j + RANK, r * 128:(r + 1) * 128],
                        aux[32 * j:32 * j + RANK, ATW:ATW + OC],
                        start=True, stop=True, tile_position=(32 * j, 0))
                lora_ps[r] = (plA, plB)

            def emit_main(t, ib):
                nc.tensor.matmul(
                    mains[t][:], xb_tiles[t][:, ib * 128:(ib + 1) * 128],
                    wad_ap(ib), start=(ib == 0), stop=(ib == NI - 1))

            emit_lora_group(0)
            for r in range(NR):
                # DVE adds for round r (2-block batched over the lora tiles)
                plA, plB = lora_ps[r]
                for half, pl in enumerate((plA, plB)):
                    k = 2 * r + half
                    w_c, l = pair_chunk[k]
                    w2 = wadp.tile([128, 2 * OC], FP16, tag="wad",
                                   name=f"wad{k}")
                    nc.vector.tensor_add(
                        w2[:], pl[:],
                        w_c[:, l * 2 * OC:(l + 1) * 2 * OC])
                    wad2[k] = w2
                    wsq = wsqp.tile([128, 2 * OC], FP16, tag="wsq",
                                    name=f"wsq{k}")
                    nc.scalar.square(wsq[:], w2[:])
                    wsq2[k] = wsq
                # PE block for round r (runs while adds of r+1 trickle)
                if r + 1 < NR:
                    emit_lora_group(r + 1)
                for t in range(3):
                    if r == JOIN[t]:
                        for ib in range(0, 4 * r + 4):
                            emit_main(t, ib)
                    elif r > JOIN[t]:
                        for ib in range(4 * r, 4 * r + 4):
                            emit_main(t, ib)
                for j in range(4):
                    k = 2 * r + j // 2
                    wsq = wsq2[k]
                    nc.tensor.matmul(
                        ps_nsq[32 * j:32 * j + 1, :],
                        ones128[:], wsq[:, (j % 2) * OC:(j % 2 + 1) * OC],
                        start=(r == 0), stop=(r == NR - 1),
                        tile_position=(0, 32 * j))

            # ---- scale = mag / sqrt(nsq), broadcast to [128, OC] ----------
            for j in range(4):
                nc.vector.tensor_copy(strip4[32 * j:32 * j + 1, :],
                                      ps_nsq[32 * j:32 * j + 1, :])
            ps_red = sp.tile([128, OC], FP32, tag="sp", name="psred")
            nc.tensor.matmul(ps_red[0:1, :], ones128[:], strip4[:],
                             start=True, stop=True)
            nrmrow = const.tile([1, OC], FP32)
            nc.scalar.sqrt(nrmrow[:], ps_red[0:1, :])
            invrow = const.tile([1, OC], FP32)
            nc.vector.reciprocal_approx_fast(invrow[:], nrmrow[:])
            srow = const.tile([1, OC], FP16)
            nc.vector.tensor_mul(srow[:], invrow[:], magn_sb[:])
            ps_b = sp.tile([128, OC], FP32, tag="sp", name="psb")
            nc.tensor.matmul(ps_b[:], onesrow[:], srow[:],
                             start=True, stop=True)
            sbc = const.tile([128, OC], FP32)
            nc.vector.tensor_copy(sbc[:], ps_b[:])

            def drain(ps_ap, t):
                if t < NB - 1:
                    o_t = outp.tile([128, OC], FP32, tag="o", name=f"o{t}")
                    nc.vector.tensor_mul(o_t[:], ps_ap, sbc[:])
                    nc.scalar.dma_start(
                        out_d[t * 128:(t + 1) * 128, :], o_t[:])
                else:
                    # split the last drain so the first half's store
                    # overlaps the second half's multiply (shorter tail)
                    for h in range(2):
                        oh = outp.tile([128, OC // 2], FP32, tag=f"oh{h}",
                                       name=f"oh{t}_{h}")
                        cs = slice(h * (OC // 2), (h + 1) * (OC // 2))
                        nc.vector.tensor_mul(oh[:], ps_ap[:, cs], sbc[:, cs])
                        nc.scalar.dma_start(
                            out_d[t * 128:(t + 1) * 128, cs], oh[:])

            for t in range(3):
                drain(mains[t][:], t)

            # ---- main GEMM: remaining 61 token tiles ----------------------
            # t3-t6 run as half-chains in the freed lora tiles (2 banks
            # each); t7.. cycle through the 3 mp banks.
            big = {}
            for t in range(3, NB):
                xb = xbp.tile([128, NI * 128], FP16, tag="xb", name=f"xb{t}")
                nc.sync.dma_start(xb[:], xb_d[t])
                xb_tiles[t] = xb
                if t < 7:
                    if t in (3, 5):
                        big[t] = lorap.tile([128, 2 * OC], FP32, tag="pl",
                                            name=f"pmL{t}")
                    base = big[t] if t in (3, 5) else big[t - 1]
                    ps_ap = base[:, (t % 2 == 0) * OC:
                                 ((t % 2 == 0) + 1) * OC]
                else:
                    ps_m = mp.tile([128, OC], FP32, tag="mp", name=f"pm{t}")
                    ps_ap = ps_m[:]
                for ib in range(NI):
                    nc.tensor.matmul(
                        ps_ap, xb[:, ib * 128:(ib + 1) * 128],
                        wad_ap(ib), start=(ib == 0), stop=(ib == NI - 1))
                drain(ps_ap, t)

    nc.compile()
    return nc


_PROGRAM = None


def _get_program():
    global _PROGRAM
    if _PROGRAM is None:
        _PROGRAM = _build_program()
    return _PROGRAM


def _prep_inputs(x, weight, lora_a_w, lora_b_w, magnitude):
    xr = np.asarray(x, dtype=np.float32).reshape(TOK, DIN)
    wr = np.asarray(weight, dtype=np.float32)
    ar = np.asarray(lora_a_w, dtype=np.float32)
    b2 = SCALING * np.asarray(lora_b_w, dtype=np.float32)

    # x token-block-major: [NB, 128 part(i%128), NI*128] per token block
    xT = xr.T.astype(np.float16)                       # [in, tok]
    xb = np.ascontiguousarray(
        xT.reshape(NI, 128, NB, 128).transpose(2, 1, 0, 3)
        .reshape(NB, 128, NI * 128))

    wT = wr.T.astype(np.float16)                       # [in, out]
    b2t = b2.T.astype(np.float16)                      # [rank, out]
    mag32 = magnitude.astype(np.float32).reshape(1, DOUT)

    # atr4: row-packed A^T.  atr4[32j + r, h*128 + c] = A[r, (4h+j)*128 + c]
    atr4 = np.zeros((4, 32, ATW), dtype=np.float16)
    Ar = ar.astype(np.float16).reshape(RANK, NR, 4, 128)   # [r, h, j, c]
    atr4[:, :RANK, :] = Ar.transpose(2, 0, 1, 3).reshape(4, RANK, ATW)
    atr4 = atr4.reshape(128, ATW)

    in_maps = []
    for cpu in range(NCORES):
        cs = slice(cpu * OC, (cpu + 1) * OC)
        # W^T partition-major: wc[p, ib*OC + o] = W^T[ib*128 + p, o]
        wc = np.ascontiguousarray(
            wT[:, cs].reshape(NI, 128, OC).transpose(1, 0, 2)
            .reshape(128, NI * OC))
        # b2n4: row-packed s*B^T replicated into the 4 row strips
        b24 = np.zeros((4, 32, OC), dtype=np.float16)
        b24[:, :RANK, :] = b2t[None, :, cs]
        aux = np.concatenate([atr4, b24.reshape(128, OC)], axis=1)
        in_maps.append({
            "xb": xb, "wc": wc,
            "aux": np.ascontiguousarray(aux),
            "magn": np.ascontiguousarray(mag32[:, cs]),
        })
    return in_maps


def kernel(x, weight, lora_a_w, lora_b_w, magnitude, _trace=False, **_kw):
    nc = _get_program()
    in_maps = _prep_inputs(x, weight, lora_a_w, lora_b_w, magnitude)
    res = run_bass_kernel_spmd(nc, in_maps, list(range(NCORES)), trace=_trace)
    out = np.concatenate([res.results[c]["out"] for c in range(NCORES)],
                         axis=1)
    if _trace:
        kernel._last_results = res
    return out.reshape(4, 2048, DOUT)


# revision 18
# speedup vs baseline: 1.0157x; 1.0157x over previous
"""DoRA linear layer on 8 TRN2 NeuronCores.

out = (magnitude / ||W + s*B@A||_row) * (x @ (W + s*B@A)^T),  s = alpha/rank = 2.

Identity used: the reference's
    dora_out + base_out = mag_norm_scale * (base_out + s * lora_out)
                        = scale_o * (x @ W_adapted^T)

Sharding: TENSOR-PARALLEL on out_dim: core k owns output columns
[512k, 512(k+1)), x replicated (streamed), W/lora_b/magnitude column-sharded.
Norm/scale computation is fully LOCAL to each core.

Schedule notes (the PE executes its queue in FIFO order, so emission order IS
the schedule; measured constants from the NTFF profile):
  * ~8.7us fixed DMA dead time at kernel start, then ~420GB/s aggregate.
  * W^T ships partition-major (8KiB DMA lines, 4 chunks of 8 blocks) instead
    of 32 x [128, OC] blocks with 1KiB lines -- the baseline's W DMA trickled
    until t=31.7us and starved the aux phase.
  * The W_ad adds (PSUM fp32 + W fp16 -> fp16) are DVE-bound at ~677ns per
    [128,512] block (PSUM reads run 1x mode).  They are batched 2 blocks per
    op over two-bank lora PSUM tiles (~1.16us/pair) and form the aux
    critical chain; everything else pipelines around them:
      - rank-16 LoRA matmuls 4-way ROW-packed (tile_position=(32j,0), K=16),
      - nsq ones-matmuls 4-way COLUMN-packed (tile_position=(0,32j), M=1),
      - W_ad squares batched on the SCALAR engine (activation Square),
      - per aux round r the PE block [lora(r+1) | main(r) | nsq(r)] is
        emitted after the adds of round r, so adds for round r+1 always run
        one round ahead of the PE and the PE never head-of-line blocks.
  * Token tiles t0-t2 join the aux rounds adaptively (t0 at r0, t1 at r2,
    t2 at r4, with catch-up bursts) so the PE stays fed while xb tiles
    arrive; the sync-queue DMA order (aux, wc0, xb0, wc1, xb1, wc2, xb2,
    wc3, xb3..) matches that schedule.
  * scale = mag/sqrt(nsq) broadcasts to [128, OC] via a K=1 ones matmul (no
    DRAM round trip); the 4 nsq strips are reduced with partition-aligned
    PSUM->SBUF copies + one ones-matmul (DVE lanes cannot cross partitions).
  * PSUM budget 8 banks: 2x2 lora ring + 3 main ring + 1 nsq/scale; after
    the aux phase tiles t3-t6 run as half-chains in the freed lora tiles,
    giving ~27us of drain slack so the scale path never stalls the PE.
  * Input DMAs issue on the sync queue, output stores on the scalar queue;
    the last tile's drain is split in halves to shorten the tail.

Host side only reshapes/transposes (layout prep), casts fp32 -> fp16
(accuracy budget is rel_err < 2e-2; fp16 gives ~4e-4), and concatenates
the per-core output column blocks.
"""

import sys

sys.path.insert(0, "/opt/trn_rl_repo")

import numpy as np

import concourse.bass as bass  # noqa: F401  (import keeps bass registered)
from concourse import bacc
import concourse.mybir as mybir
from concourse.tile import TileContext
from concourse.bass_utils import run_bass_kernel_spmd

FP32 = mybir.dt.float32
FP16 = mybir.dt.float16

NCORES = 8
TOK = 8192          # 4 * 2048 tokens total, all processed by every core
DIN = 4096
DOUT = 4096
RANK = 16
SCALING = 32.0 / 16

NI = DIN // 128      # 32 contraction blocks
OC = DOUT // NCORES  # 512 output columns per core
NB = TOK // 128      # 64 token tiles per core
CHUNK_PAIRS = [4, 4, 4, 4]   # W DMA chunk sizes in block-pairs
NR = NI // 4         # 8 aux rounds of 4 blocks
ATW = NR * 128       # atr4 width in aux const
JOIN = {0: 0, 1: 2, 2: 4}   # aux round at which early tile t joins


def _build_program():
    nc = bacc.Bacc("TRN2", target_bir_lowering=False, debug=False,
                   num_devices=NCORES)

    # x in token-block-major layout: block t -> [128 part, NI*128] contiguous
    xb_d = nc.dram_tensor("xb", [NB, 128, NI * 128], FP16,
                          kind="ExternalInput")
    # W^T partition-major: wc[p, ib*OC + o] = W^T[ib*128 + p, o]
    wc_d = nc.dram_tensor("wc", [128, NI * OC], FP16, kind="ExternalInput")
    # aux = atr4 (row-packed A^T) ++ b2n4 (row-packed s*B^T)
    aux_d = nc.dram_tensor("aux", [128, ATW + OC], FP16, kind="ExternalInput")
    magn_d = nc.dram_tensor("magn", [1, OC], FP32, kind="ExternalInput")
    out_d = nc.dram_tensor("out", [TOK, OC], FP32, kind="ExternalOutput")

    with TileContext(nc) as tc:
        with (
            tc.tile_pool(name="const", bufs=1) as const,
            tc.tile_pool(name="xbp", bufs=8) as xbp,
            tc.tile_pool(name="wadp", bufs=NI // 2) as wadp,
            tc.tile_pool(name="wsqp", bufs=2) as wsqp,
            tc.tile_pool(name="outp", bufs=10) as outp,
            tc.tile_pool(name="lorap", bufs=2, space="PSUM") as lorap,
            tc.tile_pool(name="mp", bufs=3, space="PSUM") as mp,
            tc.tile_pool(name="sp", bufs=1, space="PSUM") as sp,
        ):
            # ---- constants ------------------------------------------------
            aux = const.tile([128, ATW + OC], FP16)
            nc.sync.dma_start(aux[:], aux_d[:])
            ones128 = const.tile([128, 1], FP16)
            nc.vector.memset(ones128[:], 1.0)
            onesrow = const.tile([1, 128], FP16)
            nc.vector.memset(onesrow[:], 1.0)
            strip4 = const.tile([128, OC], FP16)
            nc.vector.memset(strip4[:], 0.0)

            # ---- input DMAs (sync queue order == emission order) ----------
            # W chunks lead with small slices so the wad add chain starts as
            # early as possible; xb tiles interleave to match tile joins.
            pair_chunk = []              # pair k -> (chunk tile, local off)
            xb_tiles = {}
            xb_after = {0: 0, 1: 1, 2: 2}   # chunk idx -> xb tile to fetch
            off = 0
            for c, szp in enumerate(CHUNK_PAIRS):
                w_c = const.tile([128, szp * 2 * OC], FP16)
                nc.sync.dma_start(
                    w_c[:], wc_d[:, off * 2 * OC:(off + szp) * 2 * OC])
                for l in range(szp):
                    pair_chunk.append((w_c, l))
                off += szp
                if c in xb_after:
                    t = xb_after[c]
                    xb = xbp.tile([128, NI * 128], FP16, tag="xb",
                                  name=f"xb{t}")
                    nc.sync.dma_start(xb[:], xb_d[t])
                    xb_tiles[t] = xb
            magn_sb = const.tile([1, OC], FP32)
            nc.sync.dma_start(magn_sb[:], magn_d[:])

            # ---- aux rounds interleaved with early-main ------------------
            ps_nsq = sp.tile([128, OC], FP32, tag="sp", name="psnsq")
            mains = [mp.tile([128, OC], FP32, tag="mp", name=f"pm{t}")
                     for t in range(3)]
            wad2 = [None] * (NI // 2)    # pair k covers blocks (2k, 2k+1)
            wsq2 = [None] * (NI // 2)
            lora_ps = {}                 # round r -> (tileA, tileB)

            def wad_ap(ib):
                return wad2[ib // 2][:, (ib % 2) * OC:(ib % 2 + 1) * OC]

            def emit_lora_group(r):
                plA = lorap.tile([128, 2 * OC], FP32, tag="pl",
                                 name=f"plA{r}")
                plB = lorap.tile([128, 2 * OC], FP32, tag="pl",
                                 name=f"plB{r}")
                for j in range(4):
                    dst = (plA if j < 2 else plB)
                    nc.tensor.matmul(
                        dst[:, (j % 2) * OC:(j % 2 + 1) * OC],
                        aux[32 * j:32 * j + RANK, r * 128:(r + 1) * 128],
                        aux[32 * j:32 * j + RANK, ATW:ATW + OC],
                        start=True, stop=True, tile_position=(32 * j, 0))
                lora_ps[r] = (plA, plB)

            def emit_main(t, ib):
                nc.tensor.matmul(
                    mains[t][:], xb_tiles[t][:, ib * 128:(ib + 1) * 128],
                    wad_ap(ib), start=(ib == 0), stop=(ib == NI - 1))

            emit_lora_group(0)
            # warmup matmuls: fill the PE-idle window between the first lora
            # group (gated on the aux DMA) and the first W_ad adds (gated on
            # the first W chunk), keeping the PE busy >3.4us so HAM
            # unthrottles the clock before the aux rounds.  The real nsq
            # chain's start=True clears their output.
            for i in range(12):
                nc.tensor.matmul(ps_nsq[0:1, :], ones128[:], strip4[:],
                                 start=True, stop=True)
            for r in range(NR):
                # DVE adds for round r (2-block batched over the lora tiles)
                plA, plB = lora_ps[r]
                for half, pl in enumerate((plA, plB)):
                    k = 2 * r + half
                    w_c, l = pair_chunk[k]
                    w2 = wadp.tile([128, 2 * OC], FP16, tag="wad",
                                   name=f"wad{k}")
                    nc.vector.tensor_add(
                        w2[:], pl[:],
                        w_c[:, l * 2 * OC:(l + 1) * 2 * OC])
                    wad2[k] = w2
                    wsq = wsqp.tile([128, 2 * OC], FP16, tag="wsq",
                                    name=f"wsq{k}")
                    nc.scalar.square(wsq[:], w2[:])
                    wsq2[k] = wsq
                # PE block for round r (runs while adds of r+1 trickle)
                if r + 1 < NR:
                    emit_lora_group(r + 1)
                for t in range(3):
                    if r == JOIN[t]:
                        for ib in range(0, 4 * r + 4):
                            emit_main(t, ib)
                    elif r > JOIN[t]:
                        for ib in range(4 * r, 4 * r + 4):
                            emit_main(t, ib)
                for j in range(4):
                    k = 2 * r + j // 2
                    wsq = wsq2[k]
                    nc.tensor.matmul(
                        ps_nsq[32 * j:32 * j + 1, :],
                        ones128[:], wsq[:, (j % 2) * OC:(j % 2 + 1) * OC],
                        start=(r == 0), stop=(r == NR - 1),
                        tile_position=(0, 32 * j))

            # ---- scale = mag / sqrt(nsq), broadcast to [128, OC] ----------
            for j in range(4):
                nc.vector.tensor_copy(strip4[32 * j:32 * j + 1, :],
                                      ps_nsq[32 * j:32 * j + 1, :])
            ps_red = sp.tile([128, OC], FP32, tag="sp", name="psred")
            nc.tensor.matmul(ps_red[0:1, :], ones128[:], strip4[:],
                             start=True, stop=True)
            nrmrow = const.tile([1, OC], FP32)
            nc.scalar.sqrt(nrmrow[:], ps_red[0:1, :])
            invrow = const.tile([1, OC], FP32)
            nc.vector.reciprocal_approx_fast(invrow[:], nrmrow[:])
            srow = const.tile([1, OC], FP16)
            nc.vector.tensor_mul(srow[:], invrow[:], magn_sb[:])
            ps_b = sp.tile([128, OC], FP32, tag="sp", name="psb")
            nc.tensor.matmul(ps_b[:], onesrow[:], srow[:],
                             start=True, stop=True)
            sbc = const.tile([128, OC], FP32)
            nc.vector.tensor_copy(sbc[:], ps_b[:])

            def drain(ps_ap, t):
                if t < NB - 1:
                    o_t = outp.tile([128, OC], FP32, tag="o", name=f"o{t}")
                    nc.vector.tensor_mul(o_t[:], ps_ap, sbc[:])
                    nc.scalar.dma_start(
                        out_d[t * 128:(t + 1) * 128, :], o_t[:])
                else:
                    # split the last drain so the first half's store
                    # overlaps the second half's multiply (shorter tail)
                    for h in range(2):
                        oh = outp.tile([128, OC // 2], FP32, tag=f"oh{h}",
                                       name=f"oh{t}_{h}")
                        cs = slice(h * (OC // 2), (h + 1) * (OC // 2))
                        nc.vector.tensor_mul(oh[:], ps_ap[:, cs], sbc[:, cs])
                        nc.scalar.dma_start(
                            out_d[t * 128:(t + 1) * 128, cs], oh[:])

            for t in range(3):
                drain(mains[t][:], t)

            # ---- main GEMM: remaining 61 token tiles ----------------------
            # t3-t6 run as half-chains in the freed lora tiles (2 banks
            # each); t7.. cycle through the 3 mp banks.
            big = {}
            for t in range(3, NB):
                xb = xbp.tile([128, NI * 128], FP16, tag="xb", name=f"xb{t}")
                nc.sync.dma_start(xb[:], xb_d[t])
                xb_tiles[t] = xb
                if t < 7:
                    if t in (3, 5):
                        big[t] = lorap.tile([128, 2 * OC], FP32, tag="pl",
                                            name=f"pmL{t}")
                    base = big[t] if t in (3, 5) else big[t - 1]
                    ps_ap = base[:, (t % 2 == 0) * OC:
                                 ((t % 2 == 0) + 1) * OC]
                else:
                    ps_m = mp.tile([128, OC], FP32, tag="mp", name=f"pm{t}")
                    ps_ap = ps_m[:]
                for ib in range(NI):
                    nc.tensor.matmul(
                        ps_ap, xb[:, ib * 128:(ib + 1) * 128],
                        wad_ap(ib), start=(ib == 0), stop=(ib == NI - 1))
                drain(ps_ap, t)

    nc.compile()
    return nc


_PROGRAM = None


def _get_program():
    global _PROGRAM
    if _PROGRAM is None:
        _PROGRAM = _build_program()
    return _PROGRAM


def _prep_inputs(x, weight, lora_a_w, lora_b_w, magnitude):
    xr = np.asarray(x, dtype=np.float32).reshape(TOK, DIN)
    wr = np.asarray(weight, dtype=np.float32)
    ar = np.asarray(lora_a_w, dtype=np.float32)
    b2 = SCALING * np.asarray(lora_b_w, dtype=np.float32)

    # x token-block-major: [NB, 128 part(i%128), NI*128] per token block
    xT = xr.T.astype(np.float16)                       # [in, tok]
    xb = np.ascontiguousarray(
        xT.reshape(NI, 128, NB, 128).transpose(2, 1, 0, 3)
        .reshape(NB, 128, NI * 128))

    wT = wr.T.astype(np.float16)                       # [in, out]
    b2t = b2.T.astype(np.float16)                      # [rank, out]
    mag32 = magnitude.astype(np.float32).reshape(1, DOUT)

    # atr4: row-packed A^T.  atr4[32j + r, h*128 + c] = A[r, (4h+j)*128 + c]
    atr4 = np.zeros((4, 32, ATW), dtype=np.float16)
    Ar = ar.astype(np.float16).reshape(RANK, NR, 4, 128)   # [r, h, j, c]
    atr4[:, :RANK, :] = Ar.transpose(2, 0, 1, 3).reshape(4, RANK, ATW)
    atr4 = atr4.reshape(128, ATW)

    in_maps = []
    for cpu in range(NCORES):
        cs = slice(cpu * OC, (cpu + 1) * OC)
        # W^T partition-major: wc[p, ib*OC + o] = W^T[ib*128 + p, o]
        wc = np.ascontiguousarray(
            wT[:, cs].reshape(NI, 128, OC).transpose(1, 0, 2)
            .reshape(128, NI * OC))
        # b2n4: row-packed s*B^T replicated into the 4 row strips
        b24 = np.zeros((4, 32, OC), dtype=np.float16)
        b24[:, :RANK, :] = b2t[None, :, cs]
        aux = np.concatenate([atr4, b24.reshape(128, OC)], axis=1)
        in_maps.append({
            "xb": xb, "wc": wc,
            "aux": np.ascontiguousarray(aux),
            "magn": np.ascontiguousarray(mag32[:, cs]),
        })
    return in_maps


def kernel(x, weight, lora_a_w, lora_b_w, magnitude, _trace=False, **_kw):
    nc = _get_program()
    in_maps = _prep_inputs(x, weight, lora_a_w, lora_b_w, magnitude)
    res = run_bass_kernel_spmd(nc, in_maps, list(range(NCORES)), trace=_trace)
    out = np.concatenate([res.results[c]["out"] for c in range(NCORES)],
                         axis=1)
    if _trace:
        kernel._last_results = res
    return out.reshape(4, 2048, DOUT)
